# revision 9
# baseline (speedup 1.0000x reference)
"""HDClassifier Trainium2 kernel (v2).

Math (per batch b):
  idx[t,c]   = clip(round((x+100)/200*200), 0, 200)
  bundled[t] = sum_c level_hv[idx[t,c]] * channel_hv[c]          # ints in [-8,8]
  gram[t',d] = prod_{i=0..3} bundled[t'+i, (d-(3-i)) mod D]      # |.| <= 4096
  sample[d]  = sum_{t'=0..124} gram[t',d]
  out        = sign(sample) @ centroid.T

Device strategy (8 cores, 4 batches each):
  - Host compacts the folded table per core: only the ~1350 (channel,level)
    keys actually used by the core's 4 batches are uploaded, padded to
    KP*256 rows (KP=6 expected) -> 6 DoubleRow k-passes instead of 7.
  - Rows carry a 3-col circular halo on the left (cols 9997..9999,0..9999),
    so the n-gram's circular d-shifts become plain free-dim offsets.
  - Phase A (chunk-major, table streamed): per 512-col chunk, per batch,
    KP fp8 DoubleRow matmuls accumulate onehot.T @ table in PSUM; Act
    drains to a per-batch fp8 bundled tile [128, 10003].
  - Phase B (per quarter, per batch): DMA-stage the partition-shifted
    bundled (sh1) and u (ush); DVE: u = bund*sh1 (fp16), gram = u*ush
    (fp16, values <= 4096 exact-ish in fp16).
  - t'-reduce: one fp16 matmul per (batch, chunk) with a ones-column
    lhsT view selecting row 20b+c of a single [80,512] f32 PSUM bank
    accumulated across all 80 matmuls -> one drain + one output DMA.
  - Host: sign + tiny [32,10000]@[10000,6] matmul.
"""

import sys

sys.path.insert(0, "/opt/trn_rl_repo")

import numpy as np

import concourse.bass as bass
import concourse.mybir as mybir
from concourse import bacc
from concourse.bass_utils import run_bass_kernel_spmd
from concourse.tile import TileContext

# Problem constants (hardcoded per contract)
NUM_LEVELS = 201
N_GRAM = 4
B, T, C, D, NUM_CLASSES = 32, 128, 8, 10000, 6
N_CORES = 8
B_LOC = B // N_CORES  # 4 batches per core
K_TOT = C * NUM_LEVELS  # 1608
HALO = N_GRAM - 1  # 3
DL = D + HALO  # 10003 local bundled width

CH = 512
NCH = 20  # out-chunk grid: 19x512 + 272 over D
CHW_B = [min(CH, D - CH * c) for c in range(NCH)]  # out-chunk widths (last 272)
# phase-A chunk grid over DL=10003: 19x512, a 3-col sliver, then 272.
# The sliver lets the 4th phase-B group (out-chunks 15-18, needing bundled
# cols <= 9730) start before the last 272-col chunk, so only the final
# 272-col group is exposed as tail.
A_STARTS = [CH * c for c in range(19)] + [9728, 9731]
A_WIDTHS = [CH] * 19 + [3, DL - 9731]
NCA = len(A_STARTS)  # 21
# phase-B groups: (first out-chunk, n chunks, ready after A-chunk index).
# Front-loaded sizes so only a tiny group is exposed as tail: a group
# ending at out-chunk e reads bundled cols <= 512(e+1)+2, i.e. A-chunk
# e+1 (the sliver for e=18, the last A-chunk for e=19).
QDEF = [
    (0, 6, 6),
    (6, 5, 11),
    (11, 4, 15),
    (15, 2, 17),
    (17, 1, 18),
    (18, 1, 19),
    (19, 1, 20),
]
QMAX = 6 * CH + 2  # largest group width (+2 overhang)

FP8 = mybir.dt.float8e4
FP16 = mybir.dt.float16
F32 = mybir.dt.float32
NP_FP8 = np.dtype(mybir.dt.np(FP8))
NP_FP16 = np.dtype(mybir.dt.np(FP16))

_CACHE = {}


def _build_program(kp):
    nc = bacc.Bacc("TRN2", target_bir_lowering=False, debug=False, num_devices=N_CORES)

    table_p = nc.declare_dram_parameter("table", [128, kp, 2, DL], FP8, isOutput=False)
    oh_p = nc.declare_dram_parameter("onehot", [128, B_LOC, kp, 2, T], FP8, isOutput=False)
    eb_p = nc.declare_dram_parameter("eb", [128, 2 * 80 + 1], FP16, isOutput=False)
    out_p = nc.declare_dram_parameter("sample", [80, CH], F32, isOutput=True)

    with TileContext(nc) as tc:
        with (
            tc.tile_pool(name="const", bufs=1) as cpool,
            tc.tile_pool(name="tab", bufs=4) as tpool,
            tc.tile_pool(name="bund", bufs=1) as bpool,
            tc.tile_pool(name="sh1", bufs=3) as shpool,
            tc.tile_pool(name="u1", bufs=3) as upool,
            tc.tile_pool(name="ush", bufs=3) as uspool,
            tc.tile_pool(name="gram", bufs=3) as gpool,
            tc.tile_pool(name="psA", bufs=6, space="PSUM") as psA_pool,
            tc.tile_pool(name="psB", bufs=1, space="PSUM") as psB_pool,
        ):
            oh_sb = cpool.tile([128, B_LOC, kp, 2, T], FP8, tag="oh")
            for b in range(B_LOC):
                nc.sync.dma_start(out=oh_sb[:, b], in_=oh_p[:, b])
            eb_sb = cpool.tile([128, 2 * 80 + 1], FP16, tag="eb")

            bund = [
                bpool.tile([128, DL], FP8, tag=f"bund{b}", name=f"bund{b}")
                for b in range(B_LOC)
            ]
            psBIG = psB_pool.tile([80, CH], F32, tag="psBIG")
            nred = [0]  # count of reduce matmuls emitted

            def phase_b_unit(q, b):
                c_first, n_ch, _ = QDEF[q]
                q0 = CH * c_first  # d-offset of group
                wq = CH * (n_ch - 1) + CHW_B[c_first + n_ch - 1]
                bd = bund[b]
                # sh1[t, j] = bund[t+1, q0+j+1], j in [0, wq+2)
                sh1 = shpool.tile([128, QMAX], FP8, tag="sh1")
                nc.sync.dma_start(
                    out=sh1[:127, 0 : wq + 2], in_=bd[1:128, q0 + 1 : q0 + wq + 3]
                )
                # u1[t, j] = bund[t, q0+j] * bund[t+1, q0+j+1]
                u1 = upool.tile([128, QMAX], FP16, tag="u1")
                nc.vector.tensor_mul(
                    out=u1[:127, 0 : wq + 2],
                    in0=bd[:127, q0 : q0 + wq + 2],
                    in1=sh1[:127, 0 : wq + 2],
                )
                # ush[p, j] = u1[p+2, j+2]
                ush = uspool.tile([128, QMAX - 2], FP16, tag="ush")
                nc.sync.dma_start(out=ush[:125, 0:wq], in_=u1[2:127, 2 : wq + 2])
                for l in range(n_ch):
                    c = c_first + l
                    w = CHW_B[c]
                    off = CH * l
                    gram = gpool.tile([128, CH], FP16, tag="gram")
                    nc.vector.tensor_mul(
                        out=gram[:125, 0:w],
                        in0=u1[:125, off : off + w],
                        in1=ush[:125, off : off + w],
                    )
                    r = b * NCH + c
                    nc.tensor.matmul(
                        psBIG[:, 0:w],
                        eb_sb[:125, 80 - r : 160 - r],
                        gram[:125, 0:w],
                        start=(nred[0] == 0),
                        stop=(nred[0] == B_LOC * NCH - 1),
                    )
                    nred[0] += 1

            # stagger phase-B units: group q's batch b is emitted after
            # A-chunk ready(q)+b so staging DMAs don't convoy ahead of
            # table-chunk DMAs (late groups clamp to the last A-chunk).
            schedule = {}
            for q, (_, _, ready) in enumerate(QDEF):
                for b in range(B_LOC):
                    schedule.setdefault(min(ready + b, NCA - 1), []).append((q, b))

            for ac in range(NCA):
                w = A_WIDTHS[ac]
                c0 = A_STARTS[ac]
                tab = tpool.tile([128, kp, 2, CH], FP8, tag="tab")
                nc.sync.dma_start(
                    out=tab[:, :, :, 0:w], in_=table_p[:, :, :, c0 : c0 + w]
                )
                if ac == 1:
                    nc.sync.dma_start(out=eb_sb[:], in_=eb_p[:])
                for b in range(B_LOC):
                    ps = psA_pool.tile([128, w], F32, tag="psA", name=f"psA{ac}_{b}")
                    for k in range(kp):
                        nc.tensor.matmul(
                            ps[:],
                            oh_sb[:, b, k, :, :],
                            tab[:, k, :, 0:w],
                            start=(k == 0),
                            stop=(k == kp - 1),
                            perf_mode=mybir.MatmulPerfMode.DoubleRow,
                        )
                    nc.scalar.copy(out=bund[b][:, c0 : c0 + w], in_=ps[:])
                for q, b in schedule.get(ac, []):
                    phase_b_unit(q, b)

            samp = cpool.tile([80, CH], F32, tag="samp")
            nc.scalar.copy(out=samp[:], in_=psBIG[:])
            nc.sync.dma_start(out=out_p[:], in_=samp[:])

    nc.finalize()
    return nc


def _host_prep(x, level_hv, channel_hv):
    # Bit-exact replication of the jax fp32 quantization
    x = np.asarray(x, dtype=np.float32)
    t1 = x + np.float32(100.0)
    t2 = t1 / np.float32(200.0)
    t3 = t2 * np.float32(200.0)
    idx = np.clip(np.rint(t3), 0, NUM_LEVELS - 1).astype(np.int32)  # [B,T,C]

    fp8_one = np.array([1.0], dtype=np.float32).astype(NP_FP8)[0]
    fp8_mone = np.array([-1.0], dtype=np.float32).astype(NP_FP8)[0]

    # folded +-1 table as fp8 bytes [1608, D]
    prod = (level_hv[None, :, :] * channel_hv[:, None, :]).reshape(K_TOT, D)
    F = np.where(prod > 0, fp8_one, fp8_mone)

    kk = np.arange(C, dtype=np.int32)[None, None, :] * NUM_LEVELS + idx  # [B,T,C]

    cores = []
    kp_max = 1
    for core in range(N_CORES):
        kk_c = kk[core * B_LOC : (core + 1) * B_LOC]  # [B_LOC, T, C]
        keys = np.unique(kk_c)
        n_k = len(keys)
        kp_c = -(-n_k // 256)
        kp_max = max(kp_max, kp_c)
        cores.append((kk_c, keys, n_k))

    kp = kp_max
    kpad = kp * 256
    in_maps = []
    eb = np.zeros((128, 2 * 80 + 1), dtype=NP_FP16)
    eb[: T - N_GRAM + 1, 80] = np.float16(1.0)
    for kk_c, keys, n_k in cores:
        inv = np.zeros(K_TOT, dtype=np.int32)
        inv[keys] = np.arange(n_k, dtype=np.int32)
        slots = inv[kk_c]  # [B_LOC, T, C]

        tabc = np.zeros((kpad, DL), dtype=NP_FP8)
        tabc[:n_k, HALO:] = F[keys]
        tabc[:n_k, :HALO] = F[keys][:, D - HALO :]
        table_up = np.ascontiguousarray(
            tabc.reshape(kp, 2, 128, DL).transpose(2, 0, 1, 3)
        )  # [128, kp, 2, DL]

        oh = np.zeros((B_LOC, kpad, T), dtype=NP_FP8)
        bb, tt, cc = np.meshgrid(
            np.arange(B_LOC), np.arange(T), np.arange(C), indexing="ij"
        )
        oh[bb.ravel(), slots.ravel(), tt.ravel()] = fp8_one
        oh_up = np.ascontiguousarray(
            oh.reshape(B_LOC, kp, 2, 128, T).transpose(3, 0, 1, 2, 4)
        )  # [128, B_LOC, kp, 2, T]

        in_maps.append({"table": table_up, "onehot": oh_up, "eb": eb})
    return kp, in_maps


def kernel(x, level_hv, channel_hv, centroid):
    kp, in_maps = _host_prep(x, level_hv, channel_hv)
    if kp not in _CACHE:
        _CACHE[kp] = _build_program(kp)
    nc = _CACHE[kp]

    res = run_bass_kernel_spmd(nc, in_maps, list(range(N_CORES)))
    _CACHE["last_results"] = res
    _CACHE["nc"] = nc

    sample = np.empty((B, D), dtype=np.float32)
    for core in range(N_CORES):
        arr = res.results[core]["sample"]  # [80, 512]
        for b in range(B_LOC):
            row = arr[b * NCH : (b + 1) * NCH]  # [20, 512]
            for c in range(NCH):
                w = CHW_B[c]
                sample[core * B_LOC + b, CH * c : CH * c + w] = row[c, :w]
    sign = np.where(sample > 0, np.float32(1.0), np.float32(-1.0))
    return (sign @ np.asarray(centroid, dtype=np.float32).T).astype(np.float32)


# revision 11
# speedup vs baseline: 1.0550x; 1.0550x over previous
"""HDClassifier Trainium2 kernel (v2).

Math (per batch b):
  idx[t,c]   = clip(round((x+100)/200*200), 0, 200)
  bundled[t] = sum_c level_hv[idx[t,c]] * channel_hv[c]          # ints in [-8,8]
  gram[t',d] = prod_{i=0..3} bundled[t'+i, (d-(3-i)) mod D]      # |.| <= 4096
  sample[d]  = sum_{t'=0..124} gram[t',d]
  out        = sign(sample) @ centroid.T

Device strategy (8 cores, 4 batches each):
  - Host compacts the folded table per core: only the ~1350 (channel,level)
    keys actually used by the core's 4 batches are uploaded, padded to
    KP*256 rows (KP=6 expected) -> 6 DoubleRow k-passes instead of 7.
  - Rows carry a 3-col circular halo on the left (cols 9997..9999,0..9999),
    so the n-gram's circular d-shifts become plain free-dim offsets.
  - Phase A (chunk-major, table streamed): per 512-col chunk, per batch,
    KP fp8 DoubleRow matmuls accumulate onehot.T @ table in PSUM; Act
    drains to a per-batch fp8 bundled tile [128, 10003].
  - Phase B (per quarter, per batch): DMA-stage the partition-shifted
    bundled (sh1) and u (ush); DVE: u = bund*sh1 (fp16), gram = u*ush
    (fp16, values <= 4096 exact-ish in fp16).
  - t'-reduce: one fp16 matmul per (batch, chunk) with a ones-column
    lhsT view selecting row 20b+c of a single [80,512] f32 PSUM bank
    accumulated across all 80 matmuls -> one drain + one output DMA.
  - Host: sign + tiny [32,10000]@[10000,6] matmul.
"""

import sys

sys.path.insert(0, "/opt/trn_rl_repo")

import numpy as np

import concourse.bass as bass
import concourse.mybir as mybir
from concourse import bacc
from concourse.bass_utils import run_bass_kernel_spmd
from concourse.tile import TileContext

# Problem constants (hardcoded per contract)
NUM_LEVELS = 201
N_GRAM = 4
B, T, C, D, NUM_CLASSES = 32, 128, 8, 10000, 6
N_CORES = 8
B_LOC = B // N_CORES  # 4 batches per core
K_TOT = C * NUM_LEVELS  # 1608
HALO = N_GRAM - 1  # 3
DL = D + HALO  # 10003 local bundled width

CH = 512
NCH = 20  # out-chunk grid: 19x512 + 272 over D
CHW_B = [min(CH, D - CH * c) for c in range(NCH)]  # out-chunk widths (last 272)
# phase-A chunk grid over DL=10003: 19x512, a 3-col sliver, then 272.
# The sliver lets the 4th phase-B group (out-chunks 15-18, needing bundled
# cols <= 9730) start before the last 272-col chunk, so only the final
# 272-col group is exposed as tail.
A_STARTS = [CH * c for c in range(19)] + [9728, 9731]
A_WIDTHS = [CH] * 19 + [3, DL - 9731]
NCA = len(A_STARTS)  # 21
# phase-B groups: (first out-chunk, n chunks, ready after A-chunk index).
# Front-loaded sizes so only a tiny group is exposed as tail: a group
# ending at out-chunk e reads bundled cols <= 512(e+1)+2, i.e. A-chunk
# e+1 (the sliver for e=18, the last A-chunk for e=19).
QDEF = [
    (0, 6, 6),
    (6, 5, 11),
    (11, 4, 15),
    (15, 2, 17),
    (17, 1, 18),
    (18, 1, 19),
    (19, 1, 20),
]
QMAX = 6 * CH + 2  # largest group width (+2 overhang)
# bundled cols < SPLIT live in fp8 tiles (cheap sh1 staging DMA, 1x DVE);
# cols >= SPLIT live in fp16 tiles (2x DVE u-mul, staging rides the
# endgame DMA slack). Groups 0-1 read fp8, groups 2+ read fp16.
SPLIT = 11 * CH  # 5632
W16 = DL - SPLIT  # 4371

FP8 = mybir.dt.float8e4
FP16 = mybir.dt.float16
F32 = mybir.dt.float32
NP_FP8 = np.dtype(mybir.dt.np(FP8))
NP_FP16 = np.dtype(mybir.dt.np(FP16))

_CACHE = {}


def _build_program(kp):
    nc = bacc.Bacc("TRN2", target_bir_lowering=False, debug=False, num_devices=N_CORES)

    table_p = nc.declare_dram_parameter("table", [128, kp, 2, DL], FP8, isOutput=False)
    oh_p = nc.declare_dram_parameter("onehot", [128, B_LOC, kp, 2, T], FP8, isOutput=False)
    eb_p = nc.declare_dram_parameter("eb", [128, 2 * 80 + 1], FP16, isOutput=False)
    out_p = nc.declare_dram_parameter("sample", [80, CH], F32, isOutput=True)

    with TileContext(nc) as tc:
        with (
            tc.tile_pool(name="const", bufs=1) as cpool,
            tc.tile_pool(name="tab", bufs=4) as tpool,
            tc.tile_pool(name="bund", bufs=1) as bpool,
            tc.tile_pool(name="sh1", bufs=3) as shpool,
            tc.tile_pool(name="sh16", bufs=3) as sh16pool,
            tc.tile_pool(name="u1", bufs=3) as upool,
            tc.tile_pool(name="ush", bufs=3) as uspool,
            tc.tile_pool(name="gram", bufs=3) as gpool,
            tc.tile_pool(name="psA", bufs=6, space="PSUM") as psA_pool,
            tc.tile_pool(name="psB", bufs=1, space="PSUM") as psB_pool,
        ):
            oh_sb = cpool.tile([128, B_LOC, kp, 2, T], FP8, tag="oh")
            nc.sync.dma_start(out=oh_sb[:, 0], in_=oh_p[:, 0])
            eb_sb = cpool.tile([128, 2 * 80 + 1], FP16, tag="eb")

            bund8 = [
                bpool.tile([128, SPLIT + CH], FP8, tag=f"bund8_{b}", name=f"b8_{b}")
                for b in range(B_LOC)
            ]
            bund16 = [
                bpool.tile([128, W16], FP16, tag=f"bund16_{b}", name=f"b16_{b}")
                for b in range(B_LOC)
            ]
            psBIG = psB_pool.tile([80, CH], F32, tag="psBIG")
            nred = [0]  # count of reduce matmuls emitted

            def phase_b_unit(q, b):
                c_first, n_ch, _ = QDEF[q]
                q0 = CH * c_first  # d-offset of group
                wq = CH * (n_ch - 1) + CHW_B[c_first + n_ch - 1]
                if q <= 1:
                    bd, q0l = bund8[b], q0
                    sh1 = shpool.tile([128, QMAX], FP8, tag="sh1")
                else:
                    bd, q0l = bund16[b], q0 - SPLIT
                    sh1 = sh16pool.tile([128, 4 * CH + 2], FP16, tag="sh16")
                # sh1[t, j] = bund[t+1, q0+j+1], j in [0, wq+2)
                nc.sync.dma_start(
                    out=sh1[:127, 0 : wq + 2], in_=bd[1:128, q0l + 1 : q0l + wq + 3]
                )
                # u1[t, j] = bund[t, q0+j] * bund[t+1, q0+j+1]
                u1 = upool.tile([128, QMAX], FP16, tag="u1")
                nc.vector.tensor_mul(
                    out=u1[:127, 0 : wq + 2],
                    in0=bd[:127, q0l : q0l + wq + 2],
                    in1=sh1[:127, 0 : wq + 2],
                )
                # ush[p, j] = u1[p+2, j+2]
                ush = uspool.tile([128, QMAX - 2], FP16, tag="ush")
                nc.sync.dma_start(out=ush[:125, 0:wq], in_=u1[2:127, 2 : wq + 2])
                for l in range(n_ch):
                    c = c_first + l
                    w = CHW_B[c]
                    off = CH * l
                    gram = gpool.tile([128, CH], FP16, tag="gram")
                    nc.vector.tensor_mul(
                        out=gram[:125, 0:w],
                        in0=u1[:125, off : off + w],
                        in1=ush[:125, off : off + w],
                    )
                    r = b * NCH + c
                    nc.tensor.matmul(
                        psBIG[:, 0:w],
                        eb_sb[:125, 80 - r : 160 - r],
                        gram[:125, 0:w],
                        start=(nred[0] == 0),
                        stop=(nred[0] == B_LOC * NCH - 1),
                    )
                    nred[0] += 1

            # stagger phase-B units: group q's batch b is emitted after
            # A-chunk ready(q)+b so staging DMAs don't convoy ahead of
            # table-chunk DMAs (late groups clamp to the last A-chunk).
            schedule = {}
            for q, (_, _, ready) in enumerate(QDEF):
                for b in range(B_LOC):
                    stag = b if q == 0 else (b if q == 1 else b // 2)
                    schedule.setdefault(min(ready + stag, NCA - 1), []).append((q, b))

            for ac in range(NCA):
                w = A_WIDTHS[ac]
                c0 = A_STARTS[ac]
                tab = tpool.tile([128, kp, 2, CH], FP8, tag="tab")
                nc.sync.dma_start(
                    out=tab[:, :, :, 0:w], in_=table_p[:, :, :, c0 : c0 + w]
                )
                if ac == 0:
                    for b in range(1, B_LOC):
                        nc.sync.dma_start(out=oh_sb[:, b], in_=oh_p[:, b])
                if ac == 1:
                    nc.sync.dma_start(out=eb_sb[:], in_=eb_p[:])
                for b in range(B_LOC):
                    ps = psA_pool.tile([128, w], F32, tag="psA", name=f"psA{ac}_{b}")
                    for k in range(kp):
                        nc.tensor.matmul(
                            ps[:],
                            oh_sb[:, b, k, :, :],
                            tab[:, k, :, 0:w],
                            start=(k == 0),
                            stop=(k == kp - 1),
                            perf_mode=mybir.MatmulPerfMode.DoubleRow,
                        )
                    if c0 < SPLIT + HALO:  # group 1's overhang reads 5632-5634
                        nc.scalar.copy(out=bund8[b][:, c0 : c0 + w], in_=ps[:])
                    if c0 + w > SPLIT:
                        lo = max(c0, SPLIT)
                        nc.scalar.copy(
                            out=bund16[b][:, lo - SPLIT : c0 + w - SPLIT],
                            in_=ps[:, lo - c0 : w],
                        )
                for q, b in schedule.get(ac, []):
                    phase_b_unit(q, b)

            samp = cpool.tile([80, CH], F32, tag="samp")
            nc.scalar.copy(out=samp[:], in_=psBIG[:])
            nc.sync.dma_start(out=out_p[:], in_=samp[:])

    nc.finalize()
    return nc


def _host_prep(x, level_hv, channel_hv):
    # Bit-exact replication of the jax fp32 quantization
    x = np.asarray(x, dtype=np.float32)
    t1 = x + np.float32(100.0)
    t2 = t1 / np.float32(200.0)
    t3 = t2 * np.float32(200.0)
    idx = np.clip(np.rint(t3), 0, NUM_LEVELS - 1).astype(np.int32)  # [B,T,C]

    fp8_one = np.array([1.0], dtype=np.float32).astype(NP_FP8)[0]
    fp8_mone = np.array([-1.0], dtype=np.float32).astype(NP_FP8)[0]

    # folded +-1 table as fp8 bytes [1608, D]
    prod = (level_hv[None, :, :] * channel_hv[:, None, :]).reshape(K_TOT, D)
    F = np.where(prod > 0, fp8_one, fp8_mone)

    kk = np.arange(C, dtype=np.int32)[None, None, :] * NUM_LEVELS + idx  # [B,T,C]

    cores = []
    kp_max = 1
    for core in range(N_CORES):
        kk_c = kk[core * B_LOC : (core + 1) * B_LOC]  # [B_LOC, T, C]
        keys = np.unique(kk_c)
        n_k = len(keys)
        kp_c = -(-n_k // 256)
        kp_max = max(kp_max, kp_c)
        cores.append((kk_c, keys, n_k))

    kp = kp_max
    kpad = kp * 256
    in_maps = []
    eb = np.zeros((128, 2 * 80 + 1), dtype=NP_FP16)
    eb[: T - N_GRAM + 1, 80] = np.float16(1.0)
    for kk_c, keys, n_k in cores:
        inv = np.zeros(K_TOT, dtype=np.int32)
        inv[keys] = np.arange(n_k, dtype=np.int32)
        slots = inv[kk_c]  # [B_LOC, T, C]

        tabc = np.zeros((kpad, DL), dtype=NP_FP8)
        tabc[:n_k, HALO:] = F[keys]
        tabc[:n_k, :HALO] = F[keys][:, D - HALO :]
        table_up = np.ascontiguousarray(
            tabc.reshape(kp, 2, 128, DL).transpose(2, 0, 1, 3)
        )  # [128, kp, 2, DL]

        oh = np.zeros((B_LOC, kpad, T), dtype=NP_FP8)
        bb, tt, cc = np.meshgrid(
            np.arange(B_LOC), np.arange(T), np.arange(C), indexing="ij"
        )
        oh[bb.ravel(), slots.ravel(), tt.ravel()] = fp8_one
        oh_up = np.ascontiguousarray(
            oh.reshape(B_LOC, kp, 2, 128, T).transpose(3, 0, 1, 2, 4)
        )  # [128, B_LOC, kp, 2, T]

        in_maps.append({"table": table_up, "onehot": oh_up, "eb": eb})
    return kp, in_maps


def kernel(x, level_hv, channel_hv, centroid):
    kp, in_maps = _host_prep(x, level_hv, channel_hv)
    if kp not in _CACHE:
        _CACHE[kp] = _build_program(kp)
    nc = _CACHE[kp]

    res = run_bass_kernel_spmd(nc, in_maps, list(range(N_CORES)))
    _CACHE["last_results"] = res
    _CACHE["nc"] = nc

    sample = np.empty((B, D), dtype=np.float32)
    for core in range(N_CORES):
        arr = res.results[core]["sample"]  # [80, 512]
        for b in range(B_LOC):
            row = arr[b * NCH : (b + 1) * NCH]  # [20, 512]
            for c in range(NCH):
                w = CHW_B[c]
                sample[core * B_LOC + b, CH * c : CH * c + w] = row[c, :w]
    sign = np.where(sample > 0, np.float32(1.0), np.float32(-1.0))
    return (sign @ np.asarray(centroid, dtype=np.float32).T).astype(np.float32)
